# revision 66
# baseline (speedup 1.0000x reference)
"""Causal multi-head attention on 8 Trainium2 NeuronCores.

Problem: B=2, S=2048, D=1024, H=16 heads (HD=64), fp32 I/O.
Sharding: batch x head-group. Core c handles batch c//4 and heads
4*(c%4) .. 4*(c%4)+3 (a 256-wide feature slice of Wq/Wk/Wv columns and
Wo rows). Each core writes a partial output projection for its batch;
the host sums the 4 partials per batch and adds the bias.

All device-side data is bf16 (host converts); matmuls accumulate in
fp32 PSUM, softmax statistics stay fp32.  At 2e-2 rel-err tolerance
the bf16 pipeline lands ~5e-3.  bf16 halves HBM traffic and enables
DVE fast modes for SBUF-only elementwise ops.

Device dataflow is fully "feature-major" (transposed) so no transposes
are ever needed on device:
  - host feeds x[b].T as xT [D, S]
  - QT = Wq_g.T @ xT (via matmul(lhsT=Wq chunk, rhs=xT chunk))  [256, S]
  - KT likewise; V in natural token-major layout via lhsT=xT chunks,
    with a ones-column appended per head (V_aug [S, 65], written by an
    on-device memset) so the ctx matmul's row 64 accumulates the
    softmax denominator for free
  - scores^T chunks [128 keys, <=512 queries] = matmul(lhsT=KT chunk,
    rhs=QT tile) with K=64 contraction; per key-chunk the two heads'
    scores are emitted back-to-back so the PE never sits in an
    exp-latency shadow
  - softmax without max-subtraction (inputs are unit-scale gaussians;
    exp cannot overflow): exp on ACT with scale=1/8 fused, causal mask
    applied as a 0/1 multiply only on diagonal-crossing chunks, fully
    masked chunks skipped entirely
  - ctx_aug^T [65, 512] accumulated over key chunks; row 64 = denom
  - normalize per head: stage the denominator row to a partition-0
    SBUF tile (plain copy; the reciprocal custom-ISA op mis-reads
    nonzero partition offsets on HW), reciprocal on DVE, broadcast
    across partitions on GpSimd, multiply on DVE -- no PE involvement,
    so tile boundaries never stall the tensor engine
  - out^T partial [1024, S] = matmul(lhsT=Wo_g chunk, rhs=ctx^T)

Load balance across engines is the main scheduling problem: the
projection work (PE-only) is front-loaded and the attention work
(ACT-heavy exp) is back-loaded because the causal last query tile
holds 40% of the softmax.  So only Q/K projections are front-loaded
(scores do not need V; V chains land just-in-time), tile 3's
projections run right after tile 0's, and tile 3's attention is
processed in four 4-chunk sweeps spread across the whole kernel, each
sweep accumulated in PSUM then folded into an SBUF fp32 accumulator
(the final sweep fused with its normalization).  Within a phase the
two head-pair streams interleave per key chunk and the ctx waves trail
the score waves so exp latency hides behind the in-order PE's queue.
Each tile's output projection streams as PE filler during the late
ACT-bound phases; the last tile's runs c-split so its first passes
overlap the final normalize, with paired output DMAs.  Engine roles:
exp on ACT, masks/broadcasts on GpSimd (SBUF-only; it cannot touch
PSUM), PSUM evictions on DVE (plus ACT at the idle tail), and a dummy
matmul warm-up burns the initial DMA wait so real work starts at full
PE clock.
"""

import numpy as np

B, S, D, H, HD = 2, 2048, 1024, 16, 64
NCORES = 8
GROUPS = 4               # head groups (cores per batch)
HPC = H // GROUPS        # heads per core = 4
DG = HPC * HD            # per-core feature width = 256
P = 128
QT = 512                 # query tile (free dim)
KC = 128                 # key chunk (partition dim)
NQT = S // QT            # 4 query tiles
NKC = S // KC            # 16 key chunks
KCH = D // P             # 8 contraction chunks for projections
MCH = DG // P            # 2 feature chunks per core (= head pairs)
OCH = D // P             # 8 output feature chunks
SWP = QT // KC           # key chunks per tile-3 sweep = 4

_compiled = None


def _build(nreps=1):
    import concourse.bass as bass
    import concourse.tile as tile
    from concourse import bacc, mybir

    f32 = mybir.dt.float32
    f32r = mybir.dt.float32r
    bf16 = mybir.dt.bfloat16
    EXP = mybir.ActivationFunctionType.Exp

    nc = bacc.Bacc("TRN2", target_bir_lowering=False, debug=False,
                   num_devices=NCORES)

    xT_d = nc.dram_tensor("xT", [D, S], bf16, kind="ExternalInput").ap()
    wq_d = nc.dram_tensor("wq", [D, DG], bf16, kind="ExternalInput").ap()
    wk_d = nc.dram_tensor("wk", [D, DG], bf16, kind="ExternalInput").ap()
    wv_d = nc.dram_tensor("wv", [D, DG], bf16, kind="ExternalInput").ap()
    wo_d = nc.dram_tensor("wo", [DG, D], bf16, kind="ExternalInput").ap()
    g_d = nc.dram_tensor("g", [P, QT + 3 * KC], bf16, kind="ExternalInput").ap()
    out_d = nc.dram_tensor("outT", [D, S], bf16, kind="ExternalOutput").ap()

    with tile.TileContext(nc) as tc:
        with tc.tile_pool(name="const", bufs=1) as const, \
             tc.tile_pool(name="work", bufs=12) as work, \
             tc.tile_pool(name="work2", bufs=3) as work2, \
             tc.tile_pool(name="outb", bufs=6) as outb, \
             tc.tile_pool(name="psA", bufs=2, space="PSUM") as psA, \
             tc.tile_pool(name="psS", bufs=2, space="PSUM") as psS, \
             tc.tile_pool(name="psC", bufs=4, space="PSUM") as psC:

            xT = const.tile([P, KCH, S], bf16, tag="xT")
            wq = const.tile([P, KCH, DG], bf16, tag="wq")
            wk = const.tile([P, KCH, DG], bf16, tag="wk")
            wv = const.tile([P, KCH, DG], bf16, tag="wv")
            wo = const.tile([P, MCH, D], bf16, tag="wo")
            g = const.tile([P, QT + 3 * KC], bf16, tag="g")
            qT = const.tile([P, MCH, S], bf16, tag="qT")
            kT = const.tile([P, MCH, S], bf16, tag="kT")
            v = const.tile([P, NKC, HPC, HD + 1], bf16, tag="v")
            ctx = const.tile([P, MCH, S], bf16, tag="ctx")
            acc3 = const.tile([HD + 1, HPC, QT], f32, tag="acc3")

            # V_aug ones column (denominator accumulator row of ctx_aug)
            nc.vector.memset(v[:, :, :, HD:HD + 1], 1.0)

            # PE warm-up: the tensor engine needs ~3us of continuous work to
            # reach full clock; burn the initial input-DMA wait on dummy
            # matmuls over a zeroed tile so the first real projection runs
            # at full speed instead of paying the p-state ramp
            warm = const.tile([P, QT], bf16, tag="warm")
            nc.vector.memset(warm[:, 0:P], 0.0)
            wps = psA.tile([P, QT], f32, tag="mm", name="warmps")
            for _ in range(11):
                nc.tensor.matmul(wps[:], lhsT=warm[:, 0:P], rhs=warm[:])

            # ---- input DMAs, ordered by first-reader time: wq + the first
            # quarter of xT tile 0 unlock the first Q-projection chain
            # ~2.2us in; tile 3's xT comes early because its projections are
            # pulled right after tile 0's (see scheduling note above).
            xTr = xT_d.rearrange("(c p) s -> p c s", p=P)
            wqr = wq_d.rearrange("(c p) n -> p c n", p=P)

            def load_xt(t):
                nc.sync.dma_start(xT[:, :, t * QT:(t + 1) * QT],
                                  xTr[:, :, t * QT:(t + 1) * QT])

            # interleave wq/xT0 chunk pairs so the first projection matmul
            # (which needs only chunk 0 of each) can start ~3.3us in
            for c0 in range(0, KCH, 2):
                nc.sync.dma_start(wq[:, c0:c0 + 2, :], wqr[:, c0:c0 + 2, :])
                nc.sync.dma_start(xT[:, c0:c0 + 2, 0:QT],
                                  xTr[:, c0:c0 + 2, 0:QT])
            nc.sync.dma_start(wk[:], wk_d.rearrange("(c p) n -> p c n", p=P))
            nc.sync.dma_start(wv[:], wv_d.rearrange("(c p) n -> p c n", p=P))
            nc.sync.dma_start(g[:], g_d[:])
            load_xt(3)
            load_xt(1)
            nc.sync.dma_start(wo[:], wo_d.rearrange("(c p) n -> p c n", p=P))
            load_xt(2)

            from collections import deque

            # Chain quanta are coarse (4 matmuls per yield) so a chain never
            # spreads across many pull points: its trailing PSUM eviction
            # would otherwise sit at the head of an in-order engine queue for
            # the whole spread, blocking every later op on that engine.
            def gen_proj_qk(w_sb, t_sb, m, t):
                ps = psA.tile([P, QT], f32, tag="mm", name="psq")
                for k in range(KCH):
                    nc.tensor.matmul(
                        ps[:],
                        lhsT=w_sb[:, k, m * P:(m + 1) * P],
                        rhs=xT[:, k, t * QT:(t + 1) * QT],
                        start=(k == 0), stop=(k == KCH - 1))
                    if k % 4 == 3:
                        yield
                nc.vector.tensor_copy(t_sb[:, m, t * QT:(t + 1) * QT], ps[:])
                yield

            def gen_proj_v(j):
                ps = psA.tile([P, QT], f32, tag="mm", name="psv")
                for k in range(KCH):
                    nc.tensor.matmul(
                        ps[:, :DG],
                        lhsT=xT[:, k, j * P:(j + 1) * P],
                        rhs=wv[:, k, :],
                        start=(k == 0), stop=(k == KCH - 1))
                    if k % 4 == 3:
                        yield
                nc.vector.tensor_copy(
                    v[:, j, :, 0:HD],
                    ps[:, :DG].rearrange("p (h d) -> p h d", h=HPC))
                yield

            def gen_proj_qk_tile(t):
                for m in range(MCH):
                    yield from gen_proj_qk(wq, qT, m, t)
                for m in range(MCH):
                    yield from gen_proj_qk(wk, kT, m, t)

            def gen_v_group(t):
                for j in range(SWP):
                    yield from gen_proj_v(t * SWP + j)

            def gen_outproj(t):
                for m in range(OCH):
                    ps = psA.tile([P, QT], f32, tag="mm", name="pso")
                    for c in range(MCH):
                        nc.tensor.matmul(
                            ps[:],
                            lhsT=wo[:, c, m * P:(m + 1) * P],
                            rhs=ctx[:, c, t * QT:(t + 1) * QT],
                            start=(c == 0), stop=(c == MCH - 1))
                        yield
                    st = outb.tile([P, QT], bf16, tag="o", name="st")
                    # GpSimd cannot read PSUM on real HW; for the last tile's
                    # era alternate ACT/DVE (ACT has slack there), else DVE
                    if t == 2 and m % 2 == 0:
                        nc.scalar.copy(st[:], ps[:])
                    else:
                        nc.vector.tensor_copy(st[:], ps[:])
                    nc.sync.dma_start(
                        out_d[m * P:(m + 1) * P, t * QT:(t + 1) * QT], st[:])
                    yield

            def pull(bg, n):
                while n > 0 and bg:
                    try:
                        next(bg[0])
                        n -= 1
                    except StopIteration:
                        bg.popleft()

            # -- scheduling state ------------------------------------------
            bgP = deque()     # projection work with a phase-end deadline
            bgO = deque()     # outproj work: spread over all future points
            remaining = [0]   # quanta left in bgP
            points = [1]      # pull points left before bgP's deadline
            oq = [0]          # quanta left in bgO
            gpts = [1]        # pull points left in the late (ACT-bound) era
            late = [False]    # outproj only drains in the late era, where
                              # the PE is otherwise starved for filler

            def add_proj(gen, quanta):
                bgP.append(gen)
                remaining[0] += quanta

            def add_outproj(t):
                bgO.append(gen_outproj(t))
                oq[0] += OCH * (MCH + 1)

            def pull1():
                n = min(-(-remaining[0] // max(points[0], 1)), remaining[0])
                remaining[0] -= n
                pull(bgP, n)
                if late[0]:
                    no = min(-(-oq[0] // max(gpts[0], 1)), oq[0])
                    oq[0] -= no
                    pull(bgO, no)
                    gpts[0] -= 1
                points[0] -= 1

            def drain_proj():
                pull(bgP, 10 ** 9)
                remaining[0] = 0

            NQK_Q = MCH * 2 * (KCH // 4 + 1)
            NV_Q = SWP * (KCH // 4 + 1)

            def attn_score(pr, hh, kc, qi, es2):
                off = HD * hh
                diag = kc >= qi * SWP
                # for a diagonal-crossing chunk, columns below w0 are fully
                # masked: skip them in scores/exp/ctx entirely, and apply the
                # triangular 0/1 mask only to the [P, KC] band at w0
                w0 = KC * (kc - qi * SWP) if diag else 0
                qlo = qi * QT + w0
                sps = psS.tile([P, QT], f32, tag="s", name="sps")
                nc.tensor.matmul(
                    sps[:, w0:],
                    lhsT=kT[off:off + HD, pr, kc * KC:(kc + 1) * KC],
                    rhs=qT[off:off + HD, pr, qlo:(qi + 1) * QT])
                es = work.tile([P, QT], bf16, tag="e", name="es")
                nc.scalar.activation(es[:, w0:], sps[:, w0:], EXP,
                                     scale=1.0 / np.sqrt(HD))
                if diag:
                    nc.gpsimd.tensor_mul(es[:, w0:w0 + KC],
                                         es[:, w0:w0 + KC],
                                         g[:, QT - KC:QT])
                es2[hh] = (es, w0)

            def attn_ctx(pr, hh, kc, qi, cps, es2, start, stop):
                es, w0 = es2[hh]
                nc.tensor.matmul(
                    cps[(pr, hh)][:, w0:],
                    lhsT=v[:, kc, 2 * pr + hh, :],
                    rhs=es[:, w0:],
                    start=start, stop=stop)

            def attn_norm(pr, qi, cps, final=False):
                # entirely off the PE (recip on DVE, broadcast on GpSimd,
                # multiply on DVE) so tile boundaries never stall the PE
                qs = slice(qi * QT, (qi + 1) * QT)
                rbs2 = {}
                for hh in range(2):
                    # per-head tiles: GpSimd APs must start at partition 0
                    dn = work2.tile([1, QT], f32, tag="d", name=f"dn{hh}")
                    if final:
                        nc.scalar.copy(dn[:], cps[(pr, hh)][HD:HD + 1, :])
                    else:
                        nc.vector.tensor_copy(dn[:], cps[(pr, hh)][HD:HD + 1, :])
                    rr = work2.tile([1, QT], f32, tag="r", name=f"rr{hh}")
                    nc.vector.reciprocal_approx_fast(rr[:], dn[:])
                    rbs2[hh] = work2.tile([HD, QT], f32, tag="rb",
                                          name=f"rbs{hh}")
                    nc.gpsimd.partition_broadcast(rbs2[hh][:], rr[:])
                for hh in range(2):
                    nc.vector.tensor_mul(
                        ctx[HD * hh:HD * (hh + 1), pr, qs],
                        cps[(pr, hh)][0:HD, :],
                        rbs2[hh][:])
                    pull1()

            def make_cps(pr, name):
                return {(pr, hh): psC.tile([HD + 1, QT], f32, tag="ctx",
                                           name=f"{name}_{pr}_{hh}")
                        for hh in range(2)}

            # The ctx waves trail the score waves by one key chunk: wave k's
            # ctx matmuls (which wait on exp) sit AFTER wave k+1's scores in
            # PE program order, so the exp stream is never starved by the
            # in-order PE, and the previous phase's normalize gets a full
            # extra wave before the psC rotation needs its tiles back.
            def attn_scores_wave(kc, qi, es_store):
                es4 = {}
                for pr in range(MCH):
                    es2 = {}
                    attn_score(pr, 0, kc, qi, es2)
                    attn_score(pr, 1, kc, qi, es2)
                    es4[pr] = es2
                    if pr == 0:
                        pull1()
                es_store[kc] = es4

            def attn_ctx_wave(kc, qi, nkc0, cps, es_store):
                es4 = es_store.pop(kc)
                start, stop = kc % nkc0 == 0, kc % nkc0 == nkc0 - 1
                for pr in range(MCH):
                    attn_ctx(pr, 0, kc, qi, cps, es4[pr], start, stop)
                    attn_ctx(pr, 1, kc, qi, cps, es4[pr], start, stop)
                    if pr == 0:
                        pull1()
                pull1()

            def attn_tile(t, bgV=None, final=False):
                nkc = (t + 1) * SWP
                cps = {}
                for pr in range(MCH):
                    cps.update(make_cps(pr, f"ctx{t}"))
                es_store = {}
                lag = 2 if nkc > 2 else 1
                for kc in range(nkc):
                    attn_scores_wave(kc, t, es_store)
                    if kc >= lag:
                        if bgV is not None and kc - lag < SWP:
                            # tile 0: the V sub-chain for this key chunk
                            # must be emitted before its ctx matmuls
                            pull(bgV, KCH // 4 + 1)
                        attn_ctx_wave(kc - lag, t, nkc, cps, es_store)
                for kc in range(nkc - lag, nkc):
                    if bgV is not None and kc < SWP:
                        pull(bgV, KCH // 4 + 1)
                    attn_ctx_wave(kc, t, nkc, cps, es_store)
                if final:
                    # start the c0 contraction passes right after head-pair
                    # 0's normalize so they overlap head-pair 1's chain
                    attn_norm(0, t, cps, final)
                    held = {m: op_c0(m, t) for m in range(4)}
                    attn_norm(1, t, cps, final)
                    outproj_final(t, held)
                else:
                    for pr in range(MCH):
                        attn_norm(pr, t, cps, final)
                    add_outproj(t)

            def op_c0(m, t):
                # borrow the (now idle) score banks: 4 chunks in flight
                qs = slice(t * QT, (t + 1) * QT)
                pool_ = psA if (m // 2) % 2 == 0 else psS
                ps = pool_.tile([P, QT], f32,
                                tag="mm" if pool_ is psA else "s",
                                name=f"psf{m}")
                nc.tensor.matmul(ps[:], lhsT=wo[:, 0, m * P:(m + 1) * P],
                                 rhs=ctx[:, 0, qs], start=True, stop=False)
                return ps

            def outproj_final(t, held):
                # program-tail output projection: c0 contraction passes
                # (needing only head-pair 0's ctx) lead the c1 passes so
                # they overlap head-pair 1's normalize chain, and the PSUM
                # evictions alternate ACT/DVE (both idle by now)
                qs = slice(t * QT, (t + 1) * QT)
                out_r = out_d.rearrange("(a p) s -> p a s", p=P)
                st2 = None
                for m in range(OCH):
                    ps = held.pop(m)
                    nc.tensor.matmul(ps[:],
                                     lhsT=wo[:, 1, m * P:(m + 1) * P],
                                     rhs=ctx[:, 1, qs],
                                     start=False, stop=True)
                    # pair adjacent chunks into one staging tile and one DMA
                    # so the final drain pays half the per-DMA overhead
                    if m % 2 == 0:
                        st2 = outb.tile([P, 2, QT], bf16, tag="o2", name="stf")
                        nc.scalar.copy(st2[:, 0, :], ps[:])
                    else:
                        nc.vector.tensor_copy(st2[:, 1, :], ps[:])
                        nc.sync.dma_start(out_r[:, m - 1:m + 1, qs], st2[:])
                    if m + 4 < OCH:
                        held[m + 4] = op_c0(m + 4, t)
                    pull(bgO, 2)

            def attn3_sweep(s):
                cps = {}
                for pr in range(MCH):
                    cps.update(make_cps(pr, f"swp{s}"))
                es_store = {}
                for j in range(SWP):
                    attn_scores_wave(s * SWP + j, 3, es_store)
                    if j >= 2:
                        attn_ctx_wave(s * SWP + j - 2, 3, SWP, cps, es_store)
                for j in range(SWP - 2, SWP):
                    attn_ctx_wave(s * SWP + j, 3, SWP, cps, es_store)
                qs3 = slice(3 * QT, 4 * QT)
                for pr in range(MCH):
                    for hh in range(2):
                        # fold the sweep into the fp32 SBUF accumulator
                        h = 2 * pr + hh
                        if s == 0:
                            nc.vector.tensor_copy(acc3[:, h, :],
                                                  cps[(pr, hh)][:])
                        else:
                            nc.vector.tensor_add(acc3[:, h, :], acc3[:, h, :],
                                                 cps[(pr, hh)][:])
                        if s == SWP - 1:
                            # final sweep: this head's accumulator is now
                            # complete, normalize it immediately (dn on ACT,
                            # recip/mult on DVE, broadcast on GpSimd) so the
                            # chains pipeline across engines and tile 3's
                            # output projection can stream as filler for the
                            # following attention phase
                            dn = work2.tile([1, QT], f32, tag="d",
                                            name=f"dn3{h}")
                            nc.scalar.copy(dn[:], acc3[HD:HD + 1, h, :])
                            rr = work2.tile([1, QT], f32, tag="r",
                                            name=f"rr3{h}")
                            nc.vector.reciprocal_approx_fast(rr[:], dn[:])
                            rbs = work2.tile([HD, QT], f32, tag="rb",
                                             name=f"rbs3{h}")
                            nc.gpsimd.partition_broadcast(rbs[:], rr[:])
                            nc.vector.tensor_mul(
                                ctx[HD * hh:HD * (hh + 1), pr, qs3],
                                acc3[0:HD, h, :], rbs[:])
                        pull1()
                if s == SWP - 1:
                    add_outproj(3)

            def npts_tile(t):
                return 3 * (t + 1) * SWP + MCH * 2

            NPTS_SWEEP = 3 * SWP + MCH * 2

            def phases():
                # Phase schedule (emission order IS program order):
                #   qk0 | attn0 [qk3, v0-3 jit] | sweep0 [qk1, v4-7]
                #   | sweep1 [qk2, v8-11] | attn1 | sweep2 | attn2 [v12-15]
                #   | sweep3 | norm3 | drain
                # QK projections run as early as possible (scores only need
                # Q/K); V chains land just before the first ctx that reads
                # them.  outproj quanta spread over the late-era points.
                gpts[0] = NPTS_SWEEP * 2 + npts_tile(2)

                for _ in gen_proj_qk_tile(0):
                    pass

                bgV = deque([gen_proj_v(j) for j in range(SWP)])
                add_proj(gen_proj_qk_tile(3), NQK_Q)
                points[0] = npts_tile(0)
                attn_tile(0, bgV=bgV)
                drain_proj()

                add_proj(gen_proj_qk_tile(1), NQK_Q)
                add_proj(gen_v_group(1), NV_Q)
                points[0] = NPTS_SWEEP
                attn3_sweep(0)
                drain_proj()

                # qk2/v8-11 are first read by sweep2: spread them across
                # BOTH sweep1 and attn1 so attn1's ACT-bound waves get fill
                add_proj(gen_proj_qk_tile(2), NQK_Q)
                add_proj(gen_v_group(2), NV_Q)
                points[0] = NPTS_SWEEP + npts_tile(1)
                attn3_sweep(1)
                attn_tile(1)
                drain_proj()

                add_proj(gen_v_group(3), NV_Q)
                late[0] = True
                points[0] = NPTS_SWEEP
                attn3_sweep(2)
                drain_proj()

                points[0] = NPTS_SWEEP
                attn3_sweep(3)

                points[0] = npts_tile(2)
                attn_tile(2, final=True)
                pull(bgO, 10 ** 9)

            for _ in range(nreps):
                phases()

    nc.compile()
    return nc


def _mask():
    # G[k, j] = 1.0 iff k <= j - (QT - KC); slice [*, goff:goff+QT] gives
    # the 0/1 causal mask for a key chunk at relative offset crel within
    # a query tile: keep iff k + KC*crel <= q.
    j = np.arange(QT + 3 * KC)[None, :]
    k = np.arange(P)[:, None]
    return (k <= j - (QT - KC))


def _in_maps(x, Wq, Wk, Wv, Wo):
    import ml_dtypes
    bf16 = ml_dtypes.bfloat16
    G = _mask().astype(bf16)
    maps = []
    for c in range(NCORES):
        b, gidx = divmod(c, GROUPS)
        sl = slice(gidx * DG, (gidx + 1) * DG)
        maps.append({
            "xT": np.ascontiguousarray(x[b].T).astype(bf16),
            "wq": np.ascontiguousarray(Wq[:, sl]).astype(bf16),
            "wk": np.ascontiguousarray(Wk[:, sl]).astype(bf16),
            "wv": np.ascontiguousarray(Wv[:, sl]).astype(bf16),
            "wo": np.ascontiguousarray(Wo[sl, :]).astype(bf16),
            "g": G,
        })
    return maps


def kernel(x, Wq, Wk, Wv, Wo, bo):
    global _compiled
    from concourse.bass_utils import run_bass_kernel_spmd

    x = np.asarray(x, dtype=np.float32)
    Wq = np.asarray(Wq, dtype=np.float32)
    Wk = np.asarray(Wk, dtype=np.float32)
    Wv = np.asarray(Wv, dtype=np.float32)
    Wo = np.asarray(Wo, dtype=np.float32)
    bo = np.asarray(bo, dtype=np.float32)

    if _compiled is None:
        _compiled = _build()
    nc = _compiled

    res = run_bass_kernel_spmd(nc, _in_maps(x, Wq, Wk, Wv, Wo),
                               list(range(NCORES)))
    out = np.zeros((B, S, D), dtype=np.float32)
    for c in range(NCORES):
        out[c // GROUPS] += res.results[c]["outT"].astype(np.float32).T
    out += bo
    return out


# revision 70
# speedup vs baseline: 1.0073x; 1.0073x over previous
"""Causal multi-head attention on 8 Trainium2 NeuronCores.

Problem: B=2, S=2048, D=1024, H=16 heads (HD=64), fp32 I/O.
Sharding: batch x head-group. Core c handles batch c//4 and heads
4*(c%4) .. 4*(c%4)+3 (a 256-wide feature slice of Wq/Wk/Wv columns and
Wo rows). Each core writes a partial output projection for its batch;
the host sums the 4 partials per batch and adds the bias.

All device-side data is bf16 (host converts); matmuls accumulate in
fp32 PSUM, softmax statistics stay fp32.  At 2e-2 rel-err tolerance
the bf16 pipeline lands ~5e-3.  bf16 halves HBM traffic and enables
DVE fast modes for SBUF-only elementwise ops.

Device dataflow is fully "feature-major" (transposed) so no transposes
are ever needed on device:
  - host feeds x[b].T as xT [D, S]
  - QT = Wq_g.T @ xT (via matmul(lhsT=Wq chunk, rhs=xT chunk))  [256, S]
  - KT likewise; V in natural token-major layout via lhsT=xT chunks,
    with a ones-column appended per head (V_aug [S, 65], written by an
    on-device memset) so the ctx matmul's row 64 accumulates the
    softmax denominator for free
  - scores^T chunks [128 keys, <=512 queries] = matmul(lhsT=KT chunk,
    rhs=QT tile) with K=64 contraction; per key-chunk the two heads'
    scores are emitted back-to-back so the PE never sits in an
    exp-latency shadow
  - softmax without max-subtraction (inputs are unit-scale gaussians;
    exp cannot overflow): exp on ACT with scale=1/8 fused, causal mask
    applied as a 0/1 multiply only on diagonal-crossing chunks, fully
    masked chunks skipped entirely
  - ctx_aug^T [65, 512] accumulated over key chunks; row 64 = denom
  - normalize per head: stage the denominator row to a partition-0
    SBUF tile (plain copy; the reciprocal custom-ISA op mis-reads
    nonzero partition offsets on HW), reciprocal on DVE, broadcast
    across partitions on GpSimd, multiply on DVE -- no PE involvement,
    so tile boundaries never stall the tensor engine
  - out^T partial [1024, S] = matmul(lhsT=Wo_g chunk, rhs=ctx^T)

Load balance across engines is the main scheduling problem: the
projection work (PE-only) is front-loaded and the attention work
(ACT-heavy exp) is back-loaded because the causal last query tile
holds 40% of the softmax.  So only Q/K projections are front-loaded
(scores do not need V; V chains land just-in-time), tile 3's
projections run right after tile 0's, and tile 3's attention is
processed in sweeps (4+4+8 key chunks) spread across the whole
kernel, each accumulated in PSUM then folded into an SBUF fp32
accumulator (the final sweep fused with its normalization).  Within a phase the
two head-pair streams interleave per key chunk and the ctx waves trail
the score waves so exp latency hides behind the in-order PE's queue.
Each tile's output projection streams as PE filler during the late
ACT-bound phases; the last tile's runs c-split so its first passes
overlap the final normalize, with paired output DMAs.  Engine roles:
exp on ACT, masks/broadcasts on GpSimd (SBUF-only; it cannot touch
PSUM), PSUM evictions on DVE (plus ACT at the idle tail), and a dummy
matmul warm-up burns the initial DMA wait so real work starts at full
PE clock.
"""

import numpy as np

B, S, D, H, HD = 2, 2048, 1024, 16, 64
NCORES = 8
GROUPS = 4               # head groups (cores per batch)
HPC = H // GROUPS        # heads per core = 4
DG = HPC * HD            # per-core feature width = 256
P = 128
QT = 512                 # query tile (free dim)
KC = 128                 # key chunk (partition dim)
NQT = S // QT            # 4 query tiles
NKC = S // KC            # 16 key chunks
KCH = D // P             # 8 contraction chunks for projections
MCH = DG // P            # 2 feature chunks per core (= head pairs)
OCH = D // P             # 8 output feature chunks
SWP = QT // KC           # key chunks per tile-3 sweep = 4

_compiled = None


def _build(nreps=1):
    import concourse.bass as bass
    import concourse.tile as tile
    from concourse import bacc, mybir

    f32 = mybir.dt.float32
    f32r = mybir.dt.float32r
    bf16 = mybir.dt.bfloat16
    EXP = mybir.ActivationFunctionType.Exp

    nc = bacc.Bacc("TRN2", target_bir_lowering=False, debug=False,
                   num_devices=NCORES)

    xT_d = nc.dram_tensor("xT", [D, S], bf16, kind="ExternalInput").ap()
    wq_d = nc.dram_tensor("wq", [D, DG], bf16, kind="ExternalInput").ap()
    wk_d = nc.dram_tensor("wk", [D, DG], bf16, kind="ExternalInput").ap()
    wv_d = nc.dram_tensor("wv", [D, DG], bf16, kind="ExternalInput").ap()
    wo_d = nc.dram_tensor("wo", [DG, D], bf16, kind="ExternalInput").ap()
    g_d = nc.dram_tensor("g", [P, QT + 3 * KC], bf16, kind="ExternalInput").ap()
    out_d = nc.dram_tensor("outT", [D, S], bf16, kind="ExternalOutput").ap()

    with tile.TileContext(nc) as tc:
        with tc.tile_pool(name="const", bufs=1) as const, \
             tc.tile_pool(name="work", bufs=12) as work, \
             tc.tile_pool(name="work2", bufs=3) as work2, \
             tc.tile_pool(name="outb", bufs=6) as outb, \
             tc.tile_pool(name="psA", bufs=2, space="PSUM") as psA, \
             tc.tile_pool(name="psS", bufs=2, space="PSUM") as psS, \
             tc.tile_pool(name="psC", bufs=4, space="PSUM") as psC:

            xT = const.tile([P, KCH, S], bf16, tag="xT")
            wq = const.tile([P, KCH, DG], bf16, tag="wq")
            wk = const.tile([P, KCH, DG], bf16, tag="wk")
            wv = const.tile([P, KCH, DG], bf16, tag="wv")
            wo = const.tile([P, MCH, D], bf16, tag="wo")
            g = const.tile([P, QT + 3 * KC], bf16, tag="g")
            qT = const.tile([P, MCH, S], bf16, tag="qT")
            kT = const.tile([P, MCH, S], bf16, tag="kT")
            v = const.tile([P, NKC, HPC, HD + 1], bf16, tag="v")
            ctx = const.tile([P, MCH, S], bf16, tag="ctx")
            acc3 = const.tile([HD + 1, HPC, QT], f32, tag="acc3")

            # V_aug ones column (denominator accumulator row of ctx_aug)
            nc.vector.memset(v[:, :, :, HD:HD + 1], 1.0)

            # PE warm-up: the tensor engine needs ~3us of continuous work to
            # reach full clock; burn the initial input-DMA wait on dummy
            # matmuls over a zeroed tile so the first real projection runs
            # at full speed instead of paying the p-state ramp
            warm = const.tile([P, QT], bf16, tag="warm")
            nc.vector.memset(warm[:, 0:P], 0.0)
            wps = psA.tile([P, QT], f32, tag="mm", name="warmps")
            for _ in range(11):
                nc.tensor.matmul(wps[:], lhsT=warm[:, 0:P], rhs=warm[:])

            # ---- input DMAs, ordered by first-reader time: wq + the first
            # quarter of xT tile 0 unlock the first Q-projection chain
            # ~2.2us in; tile 3's xT comes early because its projections are
            # pulled right after tile 0's (see scheduling note above).
            xTr = xT_d.rearrange("(c p) s -> p c s", p=P)
            wqr = wq_d.rearrange("(c p) n -> p c n", p=P)

            def load_xt(t):
                nc.sync.dma_start(xT[:, :, t * QT:(t + 1) * QT],
                                  xTr[:, :, t * QT:(t + 1) * QT])

            # interleave wq/xT0 chunk pairs so the first projection matmul
            # (which needs only chunk 0 of each) can start ~3.3us in
            for c0 in range(0, KCH, 2):
                nc.sync.dma_start(wq[:, c0:c0 + 2, :], wqr[:, c0:c0 + 2, :])
                nc.sync.dma_start(xT[:, c0:c0 + 2, 0:QT],
                                  xTr[:, c0:c0 + 2, 0:QT])
            nc.sync.dma_start(wk[:], wk_d.rearrange("(c p) n -> p c n", p=P))
            nc.sync.dma_start(wv[:], wv_d.rearrange("(c p) n -> p c n", p=P))
            nc.sync.dma_start(g[:], g_d[:])
            load_xt(3)
            load_xt(1)
            nc.sync.dma_start(wo[:], wo_d.rearrange("(c p) n -> p c n", p=P))
            load_xt(2)

            from collections import deque

            # Chain quanta are coarse (4 matmuls per yield) so a chain never
            # spreads across many pull points: its trailing PSUM eviction
            # would otherwise sit at the head of an in-order engine queue for
            # the whole spread, blocking every later op on that engine.
            def gen_proj_qk(w_sb, t_sb, m, t):
                ps = psA.tile([P, QT], f32, tag="mm", name="psq")
                for k in range(KCH):
                    nc.tensor.matmul(
                        ps[:],
                        lhsT=w_sb[:, k, m * P:(m + 1) * P],
                        rhs=xT[:, k, t * QT:(t + 1) * QT],
                        start=(k == 0), stop=(k == KCH - 1))
                    if k % 4 == 3:
                        yield
                nc.vector.tensor_copy(t_sb[:, m, t * QT:(t + 1) * QT], ps[:])
                yield

            def gen_proj_v(j):
                ps = psA.tile([P, QT], f32, tag="mm", name="psv")
                for k in range(KCH):
                    nc.tensor.matmul(
                        ps[:, :DG],
                        lhsT=xT[:, k, j * P:(j + 1) * P],
                        rhs=wv[:, k, :],
                        start=(k == 0), stop=(k == KCH - 1))
                    if k % 4 == 3:
                        yield
                nc.vector.tensor_copy(
                    v[:, j, :, 0:HD],
                    ps[:, :DG].rearrange("p (h d) -> p h d", h=HPC))
                yield

            def gen_proj_qk_tile(t):
                for m in range(MCH):
                    yield from gen_proj_qk(wq, qT, m, t)
                for m in range(MCH):
                    yield from gen_proj_qk(wk, kT, m, t)

            def gen_v_group(t):
                for j in range(SWP):
                    yield from gen_proj_v(t * SWP + j)

            def gen_outproj(t):
                for m in range(OCH):
                    ps = psA.tile([P, QT], f32, tag="mm", name="pso")
                    for c in range(MCH):
                        nc.tensor.matmul(
                            ps[:],
                            lhsT=wo[:, c, m * P:(m + 1) * P],
                            rhs=ctx[:, c, t * QT:(t + 1) * QT],
                            start=(c == 0), stop=(c == MCH - 1))
                        yield
                    st = outb.tile([P, QT], bf16, tag="o", name="st")
                    # GpSimd cannot read PSUM on real HW; for the last tile's
                    # era alternate ACT/DVE (ACT has slack there), else DVE
                    if t == 2 and m % 2 == 0:
                        nc.scalar.copy(st[:], ps[:])
                    else:
                        nc.vector.tensor_copy(st[:], ps[:])
                    nc.sync.dma_start(
                        out_d[m * P:(m + 1) * P, t * QT:(t + 1) * QT], st[:])
                    yield

            def pull(bg, n):
                while n > 0 and bg:
                    try:
                        next(bg[0])
                        n -= 1
                    except StopIteration:
                        bg.popleft()

            # -- scheduling state ------------------------------------------
            bgP = deque()     # projection work with a phase-end deadline
            bgO = deque()     # outproj work: spread over all future points
            remaining = [0]   # quanta left in bgP
            points = [1]      # pull points left before bgP's deadline
            oq = [0]          # quanta left in bgO
            gpts = [1]        # pull points left in the late (ACT-bound) era
            late = [False]    # outproj only drains in the late era, where
                              # the PE is otherwise starved for filler

            def add_proj(gen, quanta):
                bgP.append(gen)
                remaining[0] += quanta

            def add_outproj(t):
                bgO.append(gen_outproj(t))
                oq[0] += OCH * (MCH + 1)

            def pull1():
                n = min(-(-remaining[0] // max(points[0], 1)), remaining[0])
                remaining[0] -= n
                pull(bgP, n)
                if late[0]:
                    no = min(-(-oq[0] // max(gpts[0], 1)), oq[0])
                    oq[0] -= no
                    pull(bgO, no)
                    gpts[0] -= 1
                points[0] -= 1

            def drain_proj():
                pull(bgP, 10 ** 9)
                remaining[0] = 0

            NQK_Q = MCH * 2 * (KCH // 4 + 1)
            NV_Q = SWP * (KCH // 4 + 1)

            def attn_score(pr, hh, kc, qi, es2):
                off = HD * hh
                diag = kc >= qi * SWP
                # for a diagonal-crossing chunk, columns below w0 are fully
                # masked: skip them in scores/exp/ctx entirely, and apply the
                # triangular 0/1 mask only to the [P, KC] band at w0
                w0 = KC * (kc - qi * SWP) if diag else 0
                qlo = qi * QT + w0
                sps = psS.tile([P, QT], f32, tag="s", name="sps")
                nc.tensor.matmul(
                    sps[:, w0:],
                    lhsT=kT[off:off + HD, pr, kc * KC:(kc + 1) * KC],
                    rhs=qT[off:off + HD, pr, qlo:(qi + 1) * QT])
                es = work.tile([P, QT], bf16, tag="e", name="es")
                nc.scalar.activation(es[:, w0:], sps[:, w0:], EXP,
                                     scale=1.0 / np.sqrt(HD))
                if diag:
                    nc.gpsimd.tensor_mul(es[:, w0:w0 + KC],
                                         es[:, w0:w0 + KC],
                                         g[:, QT - KC:QT])
                es2[hh] = (es, w0)

            def attn_ctx(pr, hh, kc, qi, cps, es2, start, stop):
                es, w0 = es2[hh]
                nc.tensor.matmul(
                    cps[(pr, hh)][:, w0:],
                    lhsT=v[:, kc, 2 * pr + hh, :],
                    rhs=es[:, w0:],
                    start=start, stop=stop)

            def attn_norm(pr, qi, cps, final=False):
                # entirely off the PE (recip on DVE, broadcast on GpSimd,
                # multiply on DVE) so tile boundaries never stall the PE
                qs = slice(qi * QT, (qi + 1) * QT)
                rbs2 = {}
                for hh in range(2):
                    # per-head tiles: GpSimd APs must start at partition 0
                    dn = work2.tile([1, QT], f32, tag="d", name=f"dn{hh}")
                    if final:
                        nc.scalar.copy(dn[:], cps[(pr, hh)][HD:HD + 1, :])
                    else:
                        nc.vector.tensor_copy(dn[:], cps[(pr, hh)][HD:HD + 1, :])
                    rr = work2.tile([1, QT], f32, tag="r", name=f"rr{hh}")
                    nc.vector.reciprocal_approx_fast(rr[:], dn[:])
                    rbs2[hh] = work2.tile([HD, QT], f32, tag="rb",
                                          name=f"rbs{hh}")
                    nc.gpsimd.partition_broadcast(rbs2[hh][:], rr[:])
                for hh in range(2):
                    nc.vector.tensor_mul(
                        ctx[HD * hh:HD * (hh + 1), pr, qs],
                        cps[(pr, hh)][0:HD, :],
                        rbs2[hh][:])
                    pull1()

            def make_cps(pr, name):
                return {(pr, hh): psC.tile([HD + 1, QT], f32, tag="ctx",
                                           name=f"{name}_{pr}_{hh}")
                        for hh in range(2)}

            # The ctx waves trail the score waves by one key chunk: wave k's
            # ctx matmuls (which wait on exp) sit AFTER wave k+1's scores in
            # PE program order, so the exp stream is never starved by the
            # in-order PE, and the previous phase's normalize gets a full
            # extra wave before the psC rotation needs its tiles back.
            def attn_scores_wave(kc, qi, es_store):
                es4 = {}
                for pr in range(MCH):
                    es2 = {}
                    attn_score(pr, 0, kc, qi, es2)
                    attn_score(pr, 1, kc, qi, es2)
                    es4[pr] = es2
                    if pr == 0:
                        pull1()
                es_store[kc] = es4

            def attn_ctx_wave(kc, qi, nkc0, cps, es_store):
                attn_ctx_wave2(kc, qi, cps, es_store,
                               kc % nkc0 == 0, kc % nkc0 == nkc0 - 1)

            def attn_ctx_wave2(kc, qi, cps, es_store, start, stop):
                es4 = es_store.pop(kc)
                for pr in range(MCH):
                    attn_ctx(pr, 0, kc, qi, cps, es4[pr], start, stop)
                    attn_ctx(pr, 1, kc, qi, cps, es4[pr], start, stop)
                    if pr == 0:
                        pull1()
                pull1()

            def attn_tile(t, bgV=None, final=False):
                nkc = (t + 1) * SWP
                cps = {}
                for pr in range(MCH):
                    cps.update(make_cps(pr, f"ctx{t}"))
                es_store = {}
                lag = 2 if nkc > 2 else 1
                for kc in range(nkc):
                    attn_scores_wave(kc, t, es_store)
                    if kc >= lag:
                        if bgV is not None and kc - lag < SWP:
                            # tile 0: the V sub-chain for this key chunk
                            # must be emitted before its ctx matmuls
                            pull(bgV, KCH // 4 + 1)
                        attn_ctx_wave(kc - lag, t, nkc, cps, es_store)
                for kc in range(nkc - lag, nkc):
                    if bgV is not None and kc < SWP:
                        pull(bgV, KCH // 4 + 1)
                    attn_ctx_wave(kc, t, nkc, cps, es_store)
                if final:
                    # start the c0 contraction passes right after head-pair
                    # 0's normalize so they overlap head-pair 1's chain
                    attn_norm(0, t, cps, final)
                    held = {m: op_c0(m, t) for m in range(4)}
                    attn_norm(1, t, cps, final)
                    outproj_final(t, held)
                else:
                    for pr in range(MCH):
                        attn_norm(pr, t, cps, final)
                    add_outproj(t)

            def op_c0(m, t):
                # borrow the (now idle) score banks: 4 chunks in flight
                qs = slice(t * QT, (t + 1) * QT)
                pool_ = psA if (m // 2) % 2 == 0 else psS
                ps = pool_.tile([P, QT], f32,
                                tag="mm" if pool_ is psA else "s",
                                name=f"psf{m}")
                nc.tensor.matmul(ps[:], lhsT=wo[:, 0, m * P:(m + 1) * P],
                                 rhs=ctx[:, 0, qs], start=True, stop=False)
                return ps

            def outproj_final(t, held):
                # program-tail output projection: c0 contraction passes
                # (needing only head-pair 0's ctx) lead the c1 passes so
                # they overlap head-pair 1's normalize chain, and the PSUM
                # evictions alternate ACT/DVE (both idle by now)
                qs = slice(t * QT, (t + 1) * QT)
                out_r = out_d.rearrange("(a p) s -> p a s", p=P)
                st2 = None
                for m in range(OCH):
                    ps = held.pop(m)
                    nc.tensor.matmul(ps[:],
                                     lhsT=wo[:, 1, m * P:(m + 1) * P],
                                     rhs=ctx[:, 1, qs],
                                     start=False, stop=True)
                    # pair adjacent chunks into one staging tile and one DMA
                    # so the final drain pays half the per-DMA overhead
                    if m % 2 == 0:
                        st2 = outb.tile([P, 2, QT], bf16, tag="o2", name="stf")
                        nc.scalar.copy(st2[:, 0, :], ps[:])
                    else:
                        nc.vector.tensor_copy(st2[:, 1, :], ps[:])
                        nc.sync.dma_start(out_r[:, m - 1:m + 1, qs], st2[:])
                    if m + 4 < OCH:
                        held[m + 4] = op_c0(m + 4, t)
                    pull(bgO, 2)

            def attn3_sweep(k0, k1, first, last, bgV=None):
                cps = {}
                for pr in range(MCH):
                    cps.update(make_cps(pr, f"swp{k0}"))
                es_store = {}
                lag = 2 if k1 - k0 > 2 else 1

                def ctxw(kc):
                    if bgV is not None and kc >= 3 * SWP:
                        # JIT: this key chunk's V sub-chain must be emitted
                        # before its ctx matmuls
                        pull(bgV, KCH // 4 + 1)
                    attn_ctx_wave2(kc, 3, cps, es_store,
                                   kc == k0, kc == k1 - 1)

                for j, kc in enumerate(range(k0, k1)):
                    attn_scores_wave(kc, 3, es_store)
                    if j >= lag:
                        ctxw(kc - lag)
                for kc in range(k1 - lag, k1):
                    ctxw(kc)
                qs3 = slice(3 * QT, 4 * QT)
                for pr in range(MCH):
                    for hh in range(2):
                        # fold the sweep into the fp32 SBUF accumulator
                        h = 2 * pr + hh
                        if first:
                            nc.vector.tensor_copy(acc3[:, h, :],
                                                  cps[(pr, hh)][:])
                        else:
                            nc.vector.tensor_add(acc3[:, h, :], acc3[:, h, :],
                                                 cps[(pr, hh)][:])
                        if last:
                            # final sweep: this head's accumulator is now
                            # complete, normalize it immediately (dn on ACT,
                            # recip/mult on DVE, broadcast on GpSimd) so the
                            # chains pipeline across engines and tile 3's
                            # output projection can stream as filler for the
                            # following attention phase
                            dn = work2.tile([1, QT], f32, tag="d",
                                            name=f"dn3{h}")
                            nc.scalar.copy(dn[:], acc3[HD:HD + 1, h, :])
                            rr = work2.tile([1, QT], f32, tag="r",
                                            name=f"rr3{h}")
                            nc.vector.reciprocal_approx_fast(rr[:], dn[:])
                            rbs = work2.tile([HD, QT], f32, tag="rb",
                                             name=f"rbs3{h}")
                            nc.gpsimd.partition_broadcast(rbs[:], rr[:])
                            nc.vector.tensor_mul(
                                ctx[HD * hh:HD * (hh + 1), pr, qs3],
                                acc3[0:HD, h, :], rbs[:])
                        pull1()
                if last:
                    add_outproj(3)

            def npts_tile(t):
                return 3 * (t + 1) * SWP + MCH * 2

            NPTS_SWEEP = 3 * SWP + MCH * 2

            def phases():
                # Phase schedule (emission order IS program order):
                #   qk0 | attn0 [qk3, v0-3 jit] | sweep0 [qk1, v4-7]
                #   | sweep1 [qk2, v8-11] | attn1 | sweep2 | attn2 [v12-15]
                #   | sweep3 | norm3 | drain
                # QK projections run as early as possible (scores only need
                # Q/K); V chains land just before the first ctx that reads
                # them.  outproj quanta spread over the late-era points.
                gpts[0] = 3 * 2 * SWP + MCH * 2 + npts_tile(2)

                for _ in gen_proj_qk_tile(0):
                    pass

                bgV = deque([gen_proj_v(j) for j in range(SWP)])
                add_proj(gen_proj_qk_tile(3), NQK_Q)
                points[0] = npts_tile(0)
                attn_tile(0, bgV=bgV)
                drain_proj()

                add_proj(gen_proj_qk_tile(1), NQK_Q)
                add_proj(gen_v_group(1), NV_Q)
                points[0] = NPTS_SWEEP
                attn3_sweep(0, SWP, True, False)
                drain_proj()

                # qk2/v8-11 are first read by sweep2: spread them across
                # BOTH sweep1 and attn1 so attn1's ACT-bound waves get fill
                add_proj(gen_proj_qk_tile(2), NQK_Q)
                add_proj(gen_v_group(2), NV_Q)
                points[0] = NPTS_SWEEP + npts_tile(1)
                attn3_sweep(SWP, 2 * SWP, False, False)
                attn_tile(1)
                drain_proj()

                # sweeps 2+3 merged: one 8-chunk accumulation group, half
                # the folds, v12-15 chains land JIT inside it
                bgV3 = deque([gen_proj_v(j) for j in range(3 * SWP, NKC)])
                late[0] = True
                points[0] = 3 * 2 * SWP + MCH * 2
                attn3_sweep(2 * SWP, NKC, False, True, bgV=bgV3)

                points[0] = npts_tile(2)
                attn_tile(2, final=True)
                pull(bgO, 10 ** 9)

            for _ in range(nreps):
                phases()

    nc.compile()
    return nc


def _mask():
    # G[k, j] = 1.0 iff k <= j - (QT - KC); slice [*, goff:goff+QT] gives
    # the 0/1 causal mask for a key chunk at relative offset crel within
    # a query tile: keep iff k + KC*crel <= q.
    j = np.arange(QT + 3 * KC)[None, :]
    k = np.arange(P)[:, None]
    return (k <= j - (QT - KC))


def _in_maps(x, Wq, Wk, Wv, Wo):
    import ml_dtypes
    bf16 = ml_dtypes.bfloat16
    G = _mask().astype(bf16)
    maps = []
    for c in range(NCORES):
        b, gidx = divmod(c, GROUPS)
        sl = slice(gidx * DG, (gidx + 1) * DG)
        maps.append({
            "xT": np.ascontiguousarray(x[b].T).astype(bf16),
            "wq": np.ascontiguousarray(Wq[:, sl]).astype(bf16),
            "wk": np.ascontiguousarray(Wk[:, sl]).astype(bf16),
            "wv": np.ascontiguousarray(Wv[:, sl]).astype(bf16),
            "wo": np.ascontiguousarray(Wo[sl, :]).astype(bf16),
            "g": G,
        })
    return maps


def kernel(x, Wq, Wk, Wv, Wo, bo):
    global _compiled
    from concourse.bass_utils import run_bass_kernel_spmd

    x = np.asarray(x, dtype=np.float32)
    Wq = np.asarray(Wq, dtype=np.float32)
    Wk = np.asarray(Wk, dtype=np.float32)
    Wv = np.asarray(Wv, dtype=np.float32)
    Wo = np.asarray(Wo, dtype=np.float32)
    bo = np.asarray(bo, dtype=np.float32)

    if _compiled is None:
        _compiled = _build()
    nc = _compiled

    res = run_bass_kernel_spmd(nc, _in_maps(x, Wq, Wk, Wv, Wo),
                               list(range(NCORES)))
    out = np.zeros((B, S, D), dtype=np.float32)
    for c in range(NCORES):
        out[c // GROUPS] += res.results[c]["outT"].astype(np.float32).T
    out += bo
    return out
